# revision 30
# baseline (speedup 1.0000x reference)
"""Trainium2 Bass kernel for nn_AdaptiveDirectionShift.

Reference computation (B=16, C=320, H=W=64, G=5 groups of 64 channels):
  xn = zero-pad x spatially by 2          -> [B,C,68,68]
  em = mean_c(edge_guidance)              -> [B,1,64,64]
  h  = relu(conv3x3(em, w1, b1))          -> [B,16,64,64]
  dl = conv3x3(h, w2, b2)                 -> [B,2,64,64]
  dw = softmax(dl, axis=1)                -> wH = sigmoid(dl0-dl1), wW = 1-wH
  sh = roll rows of xn per group by shifts_h, crop -> [B,C,64,64]
  sw = roll cols of xn per group by shifts_w, crop
  out = wH*sh + wW*sw = sw + wH*(sh-sw)

Strategy: data-parallel over batch, 2 batches per core, no collectives.
Shift values are read host-side and baked into DMA source offsets
(compile-time specialization, like shapes).

Conv layers are im2col matmuls. The patch tensors (shifted replicas of the
padded image) are built with a few SBUF->SBUF DMAs using overlapping
[[1,3],[1,L]] source access patterns: consecutive dst partitions read the
same row stream shifted by one element. Images are stored row-padded
(66-wide rows) so all 9 taps of a 3x3 conv are plain flat offsets
66*di + dj.
"""

import numpy as np

B, C, H, W = 16, 320, 64, 64
HW = H * W
NCORES = 8
BLOC = B // NCORES  # 2 batches per core
G, CG = 5, 64       # channel groups
PAD = 2
HP = H + 2 * PAD    # 68 (padded size for roll semantics)

PL = 66             # padded line width for conv tensors
FLAT = 64 * PL      # 4224: flat length of 64 rows of 66-wide lines

PIECE = 4096
ROWS_PER_PIECE = PIECE // W  # 64
NPIECES = HW // PIECE        # 1
CTILES = [(0, 128), (128, 256), (256, 320)]
EGQ = 2048
NEGQ = HW // EGQ  # 2

LAST_RESULT = {}


def _shift_runs(s):
    """Mapping for: pad by 2, roll by s (mod 68), crop [2:66).

    dest index i in [0,64) takes src index r=(i+2-s) mod 68 of the padded
    axis; src is x[r-2] if 2<=r<66 else 0.
    Returns (data_runs, zero_runs); data_runs = list of (dst0, len, src0),
    zero_runs = list of (dst0, len).
    """
    data, zero = [], []
    cur = None
    curz = None
    for i in range(64):
        r = (i + 2 - s) % HP
        if 2 <= r < 66:
            src = r - 2
            if curz is not None:
                zero.append(curz)
                curz = None
            if cur is not None and cur[0] + cur[1] == i and cur[2] + cur[1] == src:
                cur = (cur[0], cur[1] + 1, cur[2])
            else:
                if cur is not None:
                    data.append(cur)
                cur = (i, 1, src)
        else:
            if cur is not None:
                data.append(cur)
                cur = None
            if curz is not None and curz[0] + curz[1] == i:
                curz = (curz[0], curz[1] + 1)
            else:
                if curz is not None:
                    zero.append(curz)
                curz = (i, 1)
    if cur is not None:
        data.append(cur)
    if curz is not None:
        zero.append(curz)
    return data, zero


def _build(shifts_h, shifts_w):
    from contextlib import ExitStack

    import concourse.bass as bass
    import concourse.tile as tile
    from concourse import bacc, mybir
    from concourse.tile import add_dep_helper

    f32 = mybir.dt.float32
    f32r = mybir.dt.float32r
    bf16 = mybir.dt.bfloat16
    nc = bacc.Bacc(None, target_bir_lowering=False)

    x_ext = nc.declare_dram_parameter("x", [BLOC, C, HW], f32, isOutput=False)
    eg_ext = nc.declare_dram_parameter("eg", [BLOC, C, HW], f32, isOutput=False)
    # w1t[k=3*di'+dj', co] = w1[co, di', dj'], permuted rows (bf16)
    w1t_ext = nc.declare_dram_parameter("w1t", [9, 16], bf16, isOutput=False)
    # dw2p[48*di + 3*c + dj] = (w2[0]-w2[1])[c, 3*di+dj]  (bf16)
    dw2p_ext = nc.declare_dram_parameter("dw2p", [144, 1], bf16, isOutput=False)
    ones_ext = nc.declare_dram_parameter("ones128", [128, 1], bf16, isOutput=False)
    b1_ext = nc.declare_dram_parameter("b1", [16, 1], f32, isOutput=False)
    db2_ext = nc.declare_dram_parameter("db2", [1, 1], f32, isOutput=False)
    out_ext = nc.declare_dram_parameter("out", [BLOC, C, HW], f32, isOutput=True)

    hruns = [_shift_runs(int(s)) for s in shifts_h]  # row space
    wruns = [_shift_runs(int(t)) for t in shifts_w]  # col space

    def raw_ap(tile_ap, part0, nparts, offset, free_dims):
        """AP into a tile: partitions [part0, part0+nparts), flat free-dim
        pattern starting `offset` elements into each partition."""
        pstep = tile_ap.ap[0][0]
        return bass.AP(
            tensor=tile_ap.tensor,
            offset=tile_ap.offset + pstep * part0 + offset,
            ap=[[pstep, nparts]] + [list(d) for d in free_dims],
        )

    with tile.TileContext(nc) as tc, ExitStack() as ctx:
        singles = ctx.enter_context(tc.tile_pool(name="singles", bufs=1))
        eg_pool = ctx.enter_context(tc.tile_pool(name="egp", bufs=3))
        sh_pool = ctx.enter_context(tc.tile_pool(name="shp", bufs=3))
        sw_pool = ctx.enter_context(tc.tile_pool(name="swp", bufs=3))
        d_pool = ctx.enter_context(tc.tile_pool(name="dp", bufs=2))
        egf_pool = ctx.enter_context(tc.tile_pool(name="egfp", bufs=2))
        whb_pool = ctx.enter_context(tc.tile_pool(name="whbp", bufs=2))
        gate_pool = ctx.enter_context(tc.tile_pool(name="gatep", bufs=1))
        p2c_pool = ctx.enter_context(tc.tile_pool(name="p2cp", bufs=2))
        ps_em = ctx.enter_context(tc.tile_pool(name="psem", bufs=2, space="PSUM"))
        ps_h = ctx.enter_context(tc.tile_pool(name="psh", bufs=2, space="PSUM"))
        ps_d = ctx.enter_context(tc.tile_pool(name="psd", bufs=2, space="PSUM"))

        # ---- constants ----
        ones_mean = singles.tile([128, 1], bf16, tag="ones_mean")
        nc.scalar.dma_start(out=ones_mean, in_=ones_ext[:, :])
        w1t_sb = singles.tile([9, 16], bf16, tag="w1t")
        nc.scalar.dma_start(out=w1t_sb, in_=w1t_ext[:, :])
        dw2p_sb = []
        for d in range(3):
            dwt = singles.tile([48, 1], bf16, tag=f"dw2p{d}")
            nc.scalar.dma_start(out=dwt, in_=dw2p_ext[48 * d : 48 * d + 48, :])
            dw2p_sb.append(dwt)
        b1_sb = singles.tile([16, 1], f32, tag="b1")
        nc.scalar.dma_start(out=b1_sb, in_=b1_ext[:, :])
        db2_sb = singles.tile([1, 1], f32, tag="db2")
        nc.scalar.dma_start(out=db2_sb, in_=db2_ext[:, :])

        # persistent gate tensors.
        # p1w partition 0 holds em itself (row-padded 66-wide lines);
        # partitions 1..8 hold the other 8 conv taps, in tap order
        # [0,1,2, 3,5, 6,7,8] (w1t rows are permuted to match. Engine ops
        # must start at partition 0/32/64/96, so em lives at partition 0).
        p1w = gate_pool.tile([9, FLAT], bf16, tag="p1w")
        # h_pad[c, r, cc]: 66x66 row-padded relu output, borders zero
        h_pad = gate_pool.tile([16, PL, PL], bf16, tag="h_pad")
        nc.vector.memset(h_pad, 0.0)
        wh = gate_pool.tile([1, HW], bf16, tag="wh")

        # batch-0 eg: SWDGE f32->bf16 cast loads on the gpsimd ring.
        # batch-1 eg: plain f32 loads on the scalar ring + ACT bf16 convert
        # (runs during batch-0's conv phase when ACT is idle). The two
        # streams use different rings so neither blocks the other or the
        # patch-build DMAs.
        egts = {}
        last_eg_b0 = None
        for q in range(NEGQ):
            n0 = q * EGQ
            for ct, (c0, c1) in enumerate(CTILES):
                egt = eg_pool.tile([128, EGQ], bf16, tag="egt")
                nc.gpsimd.dma_start(
                    out=egt[: c1 - c0, :], in_=eg_ext[0, c0:c1, n0 : n0 + EGQ]
                )
                egts[(0, q, ct)] = egt
        for q in range(NEGQ):
            n0 = q * EGQ
            for ct, (c0, c1) in enumerate(CTILES):
                cp = c1 - c0
                egf = egf_pool.tile([128, EGQ], f32, tag="egf")
                last_eg_b0 = nc.scalar.dma_start(
                    out=egf[:cp, :], in_=eg_ext[1, c0:c1, n0 : n0 + EGQ]
                )
                egt = eg_pool.tile([128, EGQ], bf16, tag="egt")
                nc.scalar.copy(out=egt[:cp, :], in_=egf[:cp, :])
                egts[(1, q, ct)] = egt

        whbs = []
        for b in range(BLOC):
            # ================= gate network (both batches first) ========
            # zero p1w so tap edges and em borders are zero-padded
            nc.vector.memset(p1w, 0.0)
            # channel mean of edge_guidance -> p1w partition 0 interior
            for q in range(NEGQ):
                n0 = q * EGQ
                for j in range(EGQ // 512):
                    em_ps = ps_em.tile([1, 512], f32, tag="em_ps")
                    for ct, (c0, c1) in enumerate(CTILES):
                        cp = c1 - c0
                        nc.tensor.matmul(
                            em_ps,
                            ones_mean[:cp, :],
                            egts[(b, q, ct)][:cp, j * 512 : (j + 1) * 512],
                            start=(ct == 0),
                            stop=(ct == len(CTILES) - 1),
                        )
                    r0 = (n0 + j * 512) // W
                    dst = raw_ap(p1w, 0, 1, r0 * PL + 1, [[PL, 8], [1, 64]])
                    nc.scalar.mul(
                        out=dst,
                        in_=em_ps[0:1, :].rearrange("p (r c) -> p r c", c=64),
                        mul=1.0 / C,
                    )

            # build the other 8 em patches from partition 0:
            # partition dstp+e reads the em stream at offset base + estep*e;
            # edges were pre-zeroed by the memset above.
            for dstp, np_, base, estep in (
                (1, 3, -PL - 1, 1),  # taps 0,1,2: delta -67,-66,-65
                (4, 2, -1, 2),       # taps 3,5:   delta -1,+1
                (6, 3, PL - 1, 1),   # taps 6,7,8: delta 65,66,67
            ):
                lo = max(0, -base)
                ln = FLAT - lo - max(0, base + estep * (np_ - 1))
                src = raw_ap(p1w, 0, 1, lo + base, [[estep, np_], [1, ln]])
                dst = raw_ap(p1w, dstp, np_, lo, [[1, ln]])
                nc.gpsimd.dma_start(out=dst, in_=src)

            # conv1 + relu -> h_pad interior
            for j in range(HW // 512):
                h_ps = ps_h.tile([16, 512], f32, tag="h_ps")
                r0 = (j * 512) // W
                rhs = raw_ap(p1w, 0, 9, r0 * PL + 1, [[PL, 8], [1, 64]])
                nc.tensor.matmul(
                    h_ps, w1t_sb, rhs, start=True, stop=True
                )
                nc.scalar.activation(
                    out=h_pad[0:16, 1 + r0 : 9 + r0, 1:65],
                    in_=h_ps[0:16, :].rearrange("p (r c) -> p r c", c=64),
                    func=mybir.ActivationFunctionType.Relu,
                    bias=b1_sb[0:16, 0:1],
                )

            # conv2 (single output channel = logit diff) + sigmoid -> wh.
            # Per 8-row chunk: build three K=48 patch blocks from h_pad
            # (partition 3c+dj = h stream shifted by dj, block d = row
            # offset d) with one overlapped-AP DMA each, then 3
            # accumulating matmuls.
            CLEN = 7 * PL + 64 + 1  # 527
            for j in range(HW // 512):
                d_ps = ps_d.tile([1, 512], f32, tag="d_ps")
                r0 = (j * 512) // W
                blks = []
                for d in range(3):
                    blk = p2c_pool.tile([48, CLEN], bf16, tag=f"p2c{d}")
                    base = (r0 + d) * PL - 1
                    lo = max(0, -base)
                    ln = min(CLEN - lo, PL * PL - base - 2 - lo)
                    bsrc = raw_ap(h_pad, 0, 16, lo + base, [[1, 3], [1, ln]])
                    bdst = raw_ap(blk, 0, 48, lo, [[1, ln]])
                    nc.gpsimd.dma_start(out=bdst, in_=bsrc)
                    if lo > 0:
                        nc.vector.memset(blk[0:48, 0:lo], 0.0)
                    blks.append(blk)
                for d in range(3):
                    rhs = raw_ap(blks[d], 0, 48, 1, [[PL, 8], [1, 64]])
                    nc.tensor.matmul(
                        d_ps,
                        dw2p_sb[d],
                        rhs,
                        start=(d == 0),
                        stop=(d == 2),
                    )
                nc.scalar.activation(
                    out=wh[0:1, j * 512 : (j + 1) * 512],
                    in_=d_ps[0:1, :],
                    func=mybir.ActivationFunctionType.Sigmoid,
                    bias=db2_sb[0:1, 0:1],
                )

            # broadcast wh to all 128 partitions
            whb = whb_pool.tile([128, HW], bf16, tag="whb")
            nc.gpsimd.partition_broadcast(whb, wh[0:1, :])
            whbs.append(whb)

        # ================= shifted combines (both batches) =============
        # sh loads on the sync ring, sw loads on the scalar ring, output
        # stores on the gpsimd (SWDGE) ring — three independent FIFOs, so
        # a stalled store never blocks the next tile's loads.
        first_x_inst = [None]

        def note_x(inst):
            if first_x_inst[0] is None:
                first_x_inst[0] = inst
            return inst

        for b in range(BLOC):
            whb = whbs[b]
            for ct, (c0, c1) in enumerate(CTILES):
                cp = c1 - c0
                ngroups = cp // CG
                sh_t = sh_pool.tile([cp, HW], f32, tag="sh")
                sw_t = sw_pool.tile([cp, HW], f32, tag="sw")

                for gi in range(ngroups):
                    g = ct * 2 + gi
                    p0 = gi * CG
                    ch0 = c0 + p0
                    # ---- row-shifted tile (all runs flat) ----
                    data, zero = hruns[g]
                    for (d0, ln, s0) in data:
                        note_x(nc.sync.dma_start(
                            out=sh_t[p0 : p0 + CG, d0 * W : (d0 + ln) * W],
                            in_=x_ext[b, ch0 : ch0 + CG, s0 * W : (s0 + ln) * W],
                        ))
                    for (d0, ln) in zero:
                        nc.vector.memset(
                            sh_t[p0 : p0 + CG, d0 * W : (d0 + ln) * W], 0.0
                        )
                    # ---- col-shifted tile: flat main run + fixups ----
                    t = int(shifts_w[g])
                    data, zero = wruns[g]
                    tm = ((t + 33) % HP) - 33
                    dlo = max(0, tm)
                    dhi = W + min(0, tm)
                    if dlo < dhi:
                        nc.sync.dma_start(
                            out=sw_t[p0 : p0 + CG, dlo : HW + dhi - W],
                            in_=x_ext[b, ch0 : ch0 + CG, dlo - tm : HW + dhi - W - tm],
                        )
                    sw3 = sw_t[p0 : p0 + CG, :].rearrange("p (r c) -> p r c", c=W)
                    x3 = None
                    for (d0, ln, s0) in data:
                        if d0 == dlo and s0 == d0 - tm:
                            continue  # main run already flat-copied
                        if x3 is None:
                            x3 = x_ext[b, ch0 : ch0 + CG, :].rearrange(
                                "p (r c) -> p r c", c=W
                            )
                        nc.sync.dma_start(
                            out=sw3[:, :, d0 : d0 + ln],
                            in_=x3[:, :, s0 : s0 + ln],
                        )
                    for (d0, ln) in zero:
                        nc.vector.memset(sw3[:, :, d0 : d0 + ln], 0.0)

                # ---- combine: out = sw + wh*(sh-sw) ----
                # d (bf16) frees sh's slot right after the sub; the mul runs
                # in the DVE 2x bf16 mode; the add writes f32 back into sh_t
                # which doubles as the output staging buffer.
                d_t = d_pool.tile([cp, HW], bf16, tag="d")
                nc.vector.tensor_sub(d_t, sh_t, sw_t)
                nc.vector.tensor_mul(d_t, d_t, whb[:cp, :])
                nc.vector.tensor_add(sh_t, d_t, sw_t)
                nc.scalar.dma_start(out=out_ext[b, c0:c1, :], in_=sh_t)

        if first_x_inst[0] is not None and last_eg_b0 is not None:
            add_dep_helper(
                first_x_inst[0].ins,
                last_eg_b0.ins,
                sync=True,
                reason="x prefetch waits for eg loads",
            )

    nc.finalize()
    return nc


_GRAPH_CACHE = {}


def _install_ntff_hook_shim():
    """The agent image's ``antenv`` lacks ``axon_hooks``; recreate it so
    run_bass_kernel_spmd(trace=True) can capture NTFF profiles."""
    import sys
    import types

    if "antenv.axon_hooks" in sys.modules:
        return
    try:
        import antenv
        from trn_agent_boot.trn_boot import _ntff_profile_via_ctypes
    except ImportError:
        return
    hook = _ntff_profile_via_ctypes("/opt/axon/libaxon_pjrt.so")
    mod = types.ModuleType("antenv.axon_hooks")
    mod._hook = hook

    def set_axon_ntff_profile_hook(h):
        mod._hook = h

    def get_axon_ntff_profile_hook():
        return mod._hook

    mod.set_axon_ntff_profile_hook = set_axon_ntff_profile_hook
    mod.get_axon_ntff_profile_hook = get_axon_ntff_profile_hook
    sys.modules["antenv.axon_hooks"] = mod
    antenv.axon_hooks = mod


def kernel(**inputs):
    from concourse.bass_utils import run_bass_kernel_spmd

    _install_ntff_hook_shim()

    x = np.ascontiguousarray(inputs["x"], dtype=np.float32).reshape(B, C, HW)
    eg = np.ascontiguousarray(inputs["edge_guidance"], dtype=np.float32).reshape(
        B, C, HW
    )
    w1 = np.asarray(inputs["w1"], dtype=np.float32).reshape(16, 9)
    b1 = np.asarray(inputs["b1"], dtype=np.float32).reshape(16, 1)
    w2 = np.asarray(inputs["w2"], dtype=np.float32).reshape(2, 16, 9)
    b2 = np.asarray(inputs["b2"], dtype=np.float32).reshape(2)
    shifts_h = np.asarray(inputs["shifts_h"]).astype(np.int64)
    shifts_w = np.asarray(inputs["shifts_w"]).astype(np.int64)

    import ml_dtypes

    # p1w partition order: [center tap 4, then taps 0,1,2,3,5,6,7,8]
    perm = np.array([4, 0, 1, 2, 3, 5, 6, 7, 8])
    w1t = np.ascontiguousarray(w1.T[perm]).astype(ml_dtypes.bfloat16)  # [9, 16]
    dw2 = (w2[0] - w2[1]).reshape(16, 3, 3)
    dw2p = np.ascontiguousarray(dw2.transpose(1, 0, 2).reshape(144, 1)).astype(
        ml_dtypes.bfloat16
    )  # [48*di + 3*c + dj]
    db2 = np.array([[b2[0] - b2[1]]], dtype=np.float32)

    key = (tuple(shifts_h.tolist()), tuple(shifts_w.tolist()))
    if key not in _GRAPH_CACHE:
        _GRAPH_CACHE[key] = _build(shifts_h, shifts_w)
    nc = _GRAPH_CACHE[key]

    in_maps = []
    for i in range(NCORES):
        bsl = slice(i * BLOC, (i + 1) * BLOC)
        in_maps.append(
            {
                "x": np.ascontiguousarray(x[bsl]),
                "eg": np.ascontiguousarray(eg[bsl]),
                "w1t": w1t,
                "ones128": np.ones((128, 1), ml_dtypes.bfloat16),
                "dw2p": dw2p,
                "b1": b1,
                "db2": db2,
            }
        )

    trace = True
    try:
        res = run_bass_kernel_spmd(nc, in_maps, list(range(NCORES)), trace=trace)
    except Exception:
        if not trace:
            raise
        res = run_bass_kernel_spmd(nc, in_maps, list(range(NCORES)), trace=False)

    LAST_RESULT["exec_time_ns"] = getattr(res, "exec_time_ns", None)
    LAST_RESULT["profile_json"] = getattr(res, "profile_json", None)

    out = np.concatenate([res.results[i]["out"] for i in range(NCORES)], axis=0)
    return out.reshape(B, C, H, W)
